# revision 32
# baseline (speedup 1.0000x reference)
"""AttentiveFusion Trainium2 kernel (8-core data parallel).

Reference computation per sample (B=16384 samples, NB=3 branch tokens,
D=1024, H=8 heads, HD=128):
  1. qkv = x @ in_proj_w.T            (self-attention over the 3 tokens)
  2. o   = softmax(q k^T / sqrt(HD)) v
  3. gate: w = softmax(MLP(attended.flatten()))  -> [3]
  4. weighted = sum_s w_s * attended_s
  5. out = LN(relu(LN(weighted @ r1_w.T)) @ r2_w.T)

out_w is folded into wg1 and r1 on the host (the softmax gate weights
sum to 1, so the output projection commutes with the gate-weighted sum
of branches) — the attention output o goes straight into the gate MLP
and refiner with pre-multiplied weights.

Strategy: pure data parallel over 8 NeuronCores (2048 samples each).
On each core, samples are processed in blocks of 128 (one SBUF partition
per sample for the non-matmul math).  Two phases per core:
  Phase A : qkv projection + attention -> oT [blk, p, c, s, b] (bf16)
            spilled to DRAM
  Phase BC: gating MLP + weighted sum + refiner MLP + layernorms
Matmul I/O is bf16 (fp32 accumulation in PSUM); softmax/layernorm
statistics are fp32.

All DRAM<->SBUF transfers use block-major layouts so each partition's
data per DMA is one contiguous chunk (large descriptors; small strided
descriptors measurably steal SBUF bandwidth from the PE's operand
streams and add ~20% to every matmul).
"""

import numpy as np

B, NB, D, H = 16384, 3, 1024, 8
HD = D // H
EPS = 1e-5
NCORES = 8
BC = B // NCORES          # samples per core
SB = 128                  # samples per block
P = 128

_CACHE = {}


def _np32(a):
    return np.asarray(a, dtype=np.float32)


def _build_program(n_samples):
    """Build the single-core Bass/Tile program for n_samples samples."""
    import concourse.bass as bass
    import concourse.bacc as bacc
    import concourse.mybir as mybir
    from concourse.tile import TileContext
    from concourse.masks import make_identity

    dt = mybir.dt
    AF = mybir.ActivationFunctionType
    ALU = mybir.AluOpType
    AX = mybir.AxisListType
    ts = bass.ts

    nblocks = n_samples // SB
    assert n_samples % SB == 0

    nc = bacc.Bacc("TRN2", target_bir_lowering=False, debug=False,
                   num_devices=NCORES)

    # ---- DRAM tensors (block-major: per partition contiguous) ----
    xT = nc.dram_tensor("xT", [nblocks, P, 8, NB, SB], dt.bfloat16,
                        kind="ExternalInput")
    wqkv_d = nc.dram_tensor("WqkvT", [P, 6, 8, 512], dt.bfloat16,
                            kind="ExternalInput")
    # gate L1 runs in fp8 (DoubleRow: 256-deep contraction per matmul);
    # weights are pre-scaled by G1S on the host, the relu evac divides
    # it back out.  The gate only steers a softmax over 3 branches whose
    # logits are tiny, so fp8 error is ~0.1% of the final output.
    wg1_d = nc.dram_tensor("Wg1T", [P, 2, 12, 2, 512], dt.float8e4,
                           kind="ExternalInput")
    wg2_d = nc.dram_tensor("Wg2T", [P, 8, 512], dt.bfloat16,
                           kind="ExternalInput")
    wg3_d = nc.dram_tensor("Wg3T", [P, 4, NB], dt.bfloat16,
                           kind="ExternalInput")
    r1_d = nc.dram_tensor("R1T", [P, 4, 8, 512], dt.bfloat16,
                          kind="ExternalInput")
    r2_d = nc.dram_tensor("R2T", [P, 2, 16, 512], dt.bfloat16,
                          kind="ExternalInput")
    oT_d = nc.dram_tensor("oT", [nblocks, P, 8, NB, SB], dt.bfloat16)
    oT8_d = nc.dram_tensor("oT8", [nblocks, P, 8 * NB, SB], dt.float8e4)
    out_d = nc.dram_tensor("out", [n_samples, D], dt.float32,
                           kind="ExternalOutput")
    G1S = 64.0

    from contextlib import ExitStack
    with TileContext(nc) as tc, ExitStack() as _cst:
        constp = _cst.enter_context(tc.tile_pool(name="const", bufs=1))
        ident = constp.tile([P, P], dt.bfloat16)
        make_identity(nc, ident)
        ones1 = constp.tile([1, P], dt.bfloat16)
        nc.vector.memset(ones1, 1.0)
        epst = constp.tile([P, 1], dt.float32)
        nc.vector.memset(epst, EPS)

        # Phase-BC weights prefetched during phase A (wg1 + wg2 + wg3
        # fit next to phase A's working set; r1 + r2 load at the phase
        # boundary, r1 first so bc_mid is not blocked).
        wB1 = _cst.enter_context(tc.tile_pool(name="wB1", bufs=1))
        wg18 = wB1.tile([P, 2, 12, 2, 512], dt.float8e4)
        wg2 = wB1.tile([P, 8, 512], dt.bfloat16)
        wg3 = wB1.tile([P, 4, NB], dt.bfloat16)
        # oT for blocks 0/1 stays in SBUF across the phase boundary, so
        # phase BC starts without waiting for the spill->load turnaround.
        NKEEP = 2
        okeep = _cst.enter_context(tc.tile_pool(name="okeep", bufs=1))
        keep_tiles = {}

        # ================= Phase A =================
        with tc.tile_pool(name="wA", bufs=1) as wA, \
             tc.tile_pool(name="axt", bufs=2) as pxt, \
             tc.tile_pool(name="aqkv", bufs=2) as pqkv, \
             tc.tile_pool(name="aprod", bufs=2) as pprod, \
             tc.tile_pool(name="asm", bufs=2) as psm, \
             tc.tile_pool(name="ao", bufs=3) as po, \
             tc.tile_pool(name="aoT", bufs=2) as poT, \
             tc.tile_pool(name="aoT8", bufs=2) as poT8, \
             tc.tile_pool(name="psA", bufs=4, space="PSUM") as psA:

            # wqkv on the sync queue (idle until the first o-transpose at
            # ~35us); xt loads own the scalar queue so the first matmul
            # group can start as soon as chunk 0 + xt block 0 land
            wqkv = wA.tile([P, 6, 8, 512], dt.bfloat16)
            for n in range(6):
                nc.sync.dma_start(wqkv[:, n], wqkv_d[:, n])

            def a_front(blk):
                """xt load, qkv GEMM, attention -> o (layout B)."""
                st = {"blk": blk}
                xt = pxt.tile([P, 8, NB, SB], dt.bfloat16, tag="xt")
                nc.scalar.dma_start(xt, xT[blk])
                if blk == min(2, nblocks - 1):
                    # prefetch phase-BC weights mid-phase-A
                    nc.gpsimd.dma_start(wg18, wg1_d[:])
                if blk == min(4, nblocks - 1):
                    nc.gpsimd.dma_start(wg2, wg2_d[:])
                    nc.gpsimd.dma_start(wg3, wg3_d[:])

                # qkv projection -> layout B, bf16 [128, 3, 3072]
                qkv = pqkv.tile([P, NB, 6, 512], dt.bfloat16, tag="qkv")
                for s in range(NB):
                    for n in range(6):
                        ps = psA.tile([P, 512], dt.float32, tag="psA")
                        for c in range(8):
                            nc.tensor.matmul(ps, lhsT=xt[:, c, s, :],
                                             rhs=wqkv[:, n, c, :],
                                             start=(c == 0), stop=(c == 7))
                        nc.scalar.copy(out=qkv[:, s, n, :], in_=ps)
                qv_all = qkv.rearrange("p s n e -> p s (n e)")

                # attention scores S[b, i, j, h] (h innermost so the
                # reduce writes a contiguous [P, H] run -> DVE 2x mode)
                S = psm.tile([P, NB, NB, H], dt.float32, tag="S")
                for i in range(NB):
                    qv = qv_all[:, i, 0:D].rearrange("p (h x) -> p h x", x=HD)
                    for j in range(NB):
                        kv = qv_all[:, j, D:2 * D].rearrange(
                            "p (h x) -> p h x", x=HD)
                        prod = pprod.tile([P, H, HD], dt.bfloat16, tag="prod")
                        nc.vector.tensor_mul(prod, qv, kv)
                        nc.vector.reduce_sum(out=S[:, i, j, :], in_=prod,
                                             axis=AX.X)

                # softmax over j (no max-subtraction needed: |scores| small)
                E = psm.tile([P, NB, NB, H], dt.float32, tag="E")
                nc.scalar.activation(E, S, AF.Exp)
                Z = psm.tile([P, NB, H], dt.float32, tag="Z")
                nc.vector.tensor_add(Z, E[:, :, 0, :], E[:, :, 1, :])
                nc.vector.tensor_add(Z, Z, E[:, :, 2, :])
                Zr = psm.tile([P, NB, H], dt.float32, tag="Zr")
                nc.vector.reciprocal(Zr, Z)
                attn = psm.tile([P, NB, NB, H], dt.bfloat16, tag="attn")
                nc.vector.tensor_mul(attn, E,
                                     Zr[:, :, None, :].to_broadcast(
                                         (P, NB, NB, H)))

                # o[b, i, hd, h] = sum_j attn[b,i,j,h] * v[b,j,hd,h]
                # (v columns are host-permuted to [hd, h] so the attn
                # broadcast is stride-0 on the middle dim, not innermost
                # -> DVE 2x mode; downstream weights absorb the permute)
                o = po.tile([P, NB, HD, H], dt.bfloat16, tag="o")
                for i in range(NB):
                    for j in range(NB):
                        vv = qv_all[:, j, 2 * D:3 * D].rearrange(
                            "p (x h) -> p x h", h=H)
                        a1 = attn[:, i, j, None, :].to_broadcast((P, HD, H))
                        if j == 0:
                            nc.vector.tensor_mul(o[:, i], vv, a1)
                        else:
                            tmp = pprod.tile([P, HD, H], dt.bfloat16,
                                             tag="prod")
                            nc.vector.tensor_mul(tmp, vv, a1)
                            nc.vector.tensor_add(o[:, i], o[:, i], tmp)
                st["o"] = o
                return st

            def a_back(st):
                """transpose o, spill oT (bf16) + oT8 (fp8 gate-L1 copy).
                The last blocks skip the fp8 convert here: emitted at the
                phase boundary it would sit in front of phase BC's first
                relu evacs in the in-order ACT queue (BC converts them)."""
                blk, o = st["blk"], st["o"]
                late = blk >= nblocks - 3
                if blk < NKEEP:
                    oT = okeep.tile([P, 8, NB, SB], dt.bfloat16,
                                    tag=f"keep{blk}")
                    oT8 = okeep.tile([P, 8 * NB, SB], dt.float8e4,
                                     tag=f"keep8_{blk}")
                    keep_tiles[blk] = (oT, oT8)
                else:
                    oT = poT.tile([P, 8, NB, SB], dt.bfloat16, tag="oT")
                    if not late:
                        oT8 = poT8.tile([P, 8 * NB, SB], dt.float8e4,
                                        tag="oT8")
                for s in range(NB):
                    nc.sync.dma_start_transpose(oT[:, :, s, :], o[:, s, :, :])
                if not late:
                    nc.scalar.copy(oT8, oT.rearrange("p c s b -> p (c s) b"))
                if blk >= NKEEP:
                    nc.scalar.dma_start(oT_d[blk], oT)
                    if not late:
                        nc.scalar.dma_start(oT8_d[blk], oT8)

            pending = []
            for blk in range(nblocks):
                pending.append(a_front(blk))
                if len(pending) > 2:
                    a_back(pending.pop(0))
            for stA in pending:
                a_back(stA)

        # ================= Phase BC =================
        # Software-pipelined: block N's tail (hb transposes + refiner
        # layer 2), which waits on N's LN1 chain, is emitted in the middle
        # of block N+1's work so the in-order TensorE never stalls on it.
        with tc.tile_pool(name="wB", bufs=1) as wB, \
             tc.tile_pool(name="batt", bufs=2) as patt2, \
             tc.tile_pool(name="bh1", bufs=2) as ph1, \
             tc.tile_pool(name="bh1T", bufs=2) as ph1T, \
             tc.tile_pool(name="bh2", bufs=2) as ph2, \
             tc.tile_pool(name="bw", bufs=2) as pw, \
             tc.tile_pool(name="bwt", bufs=2) as pwt, \
             tc.tile_pool(name="bhf", bufs=2) as phf, \
             tc.tile_pool(name="bhT", bufs=1) as phT, \
             tc.tile_pool(name="bout", bufs=2) as pout, \
             tc.tile_pool(name="psH1", bufs=2, space="PSUM") as psH1, \
             tc.tile_pool(name="psHF", bufs=3, space="PSUM") as psHF, \
             tc.tile_pool(name="psS", bufs=1, space="PSUM") as psS:

            # r1 first (needed first, at bc_mid of block 0), then r2; on
            # the sync queue, which is otherwise idle at the boundary
            r1 = wB.tile([P, 4, 8, 512], dt.bfloat16)
            for n in range(4):
                nc.sync.dma_start(r1[:, n], r1_d[:, n])
            r2 = wB.tile([P, 2, 16, 512], dt.bfloat16)
            nc.sync.dma_start(r2, r2_d[:])

            def bc_front(blk):
                """att load .. gate logits (+ async softmax chain)."""
                st = {"b0": blk * SB}
                b0 = st["b0"]
                if blk < NKEEP:
                    att, att8 = keep_tiles[blk]
                else:
                    att = patt2.tile([P, 8, NB, SB], dt.bfloat16, tag="att")
                    att8 = patt2.tile([P, 8 * NB, SB], dt.float8e4,
                                      tag="att8")
                    nc.scalar.dma_start(att, oT_d[blk])
                    if blk >= nblocks - 3:
                        nc.scalar.copy(att8,
                                       att.rearrange("p c s b -> p (c s) b"))
                    else:
                        nc.scalar.dma_start(att8, oT8_d[blk])
                st["att"] = att

                # gating MLP layer 1 (fp8 DoubleRow): [128, 1024]
                h1 = ph1.tile([P, D], dt.bfloat16, tag="h1")
                for n in range(2):
                    ps = psH1.tile([P, 512], dt.float32, tag="psH1")
                    for kp in range(12):
                        nc.tensor.matmul(
                            ps, lhsT=att8[:, 2 * kp:2 * kp + 2, :],
                            rhs=wg18[:, n, kp],
                            start=(kp == 0), stop=(kp == 11),
                            perf_mode=mybir.MatmulPerfMode.DoubleRow)
                    nc.scalar.activation(h1[:, ts(n, 512)], ps, AF.Relu,
                                         scale=1.0 / G1S)

                h1T = ph1T.tile([P, 8, P], dt.bfloat16, tag="h1T")
                h1v = h1.rearrange("p (c x) -> p c x", x=P)
                for g in range(2):
                    pst = psS.tile([P, 4, P], dt.bfloat16, tag="psT2")
                    for q in range(4):
                        nc.tensor.transpose(pst[:, q], h1v[:, g * 4 + q, :],
                                            ident)
                    nc.vector.tensor_copy(h1T[:, g * 4:g * 4 + 4], pst)

                # gating MLP layer 2: [128, 512]
                ps = psH1.tile([P, 512], dt.float32, tag="psH1")
                for c in range(8):
                    nc.tensor.matmul(ps, lhsT=h1T[:, c],
                                     rhs=wg2[:, c, :],
                                     start=(c == 0), stop=(c == 7))
                h2 = ph2.tile([P, D // 2], dt.bfloat16, tag="h2")
                nc.scalar.activation(h2, ps, AF.Relu)

                h2T = ph1T.tile([P, 4, P], dt.bfloat16, tag="h2T")
                h2v = h2.rearrange("p (c x) -> p c x", x=P)
                pst = psS.tile([P, 4, P], dt.bfloat16, tag="psT2")
                for q in range(4):
                    nc.tensor.transpose(pst[:, q], h2v[:, q, :], ident)
                nc.vector.tensor_copy(h2T, pst)

                # gate logits + softmax -> w [128, 3]
                psl_t = psS.tile([P, P], dt.float32, tag="psS", name="psl_t")
                psl = psl_t[:, :NB]
                for c in range(4):
                    nc.tensor.matmul(psl, lhsT=h2T[:, c], rhs=wg3[:, c],
                                     start=(c == 0), stop=(c == 3))
                Ew = pw.tile([P, NB], dt.float32, tag="Ew")
                Zw = pw.tile([P, 1], dt.float32, tag="Zw")
                nc.scalar.activation(Ew, psl, AF.Exp, accum_out=Zw)
                Zwr = pw.tile([P, 1], dt.float32, tag="Zwr")
                nc.vector.reciprocal(Zwr, Zw)
                w = pw.tile([P, NB], dt.bfloat16, tag="w")
                nc.vector.tensor_scalar_mul(w, Ew, Zwr)
                st["w"] = w
                return st

            def bc_wrow(st):
                """w column -> row via PE transpose (emitted one block
                late, so the softmax chain has a whole block to finish)."""
                w = st["w"]
                wrow = pw.tile([1, NB, P], dt.bfloat16, tag="wrow")
                for s in range(NB):
                    prt_t = psS.tile([P, P], dt.float32, tag="psS",
                                     name="prt_t")
                    prt = prt_t[:1]
                    nc.tensor.matmul(prt, lhsT=w[:, s:s + 1], rhs=ident,
                                     start=True, stop=True)
                    nc.vector.tensor_copy(wrow[:, s], prt)
                st["wrow"] = wrow

            def bc_wb_wt(st):
                """broadcast wrow to all partitions + weighted sum."""
                att, wrow = st["att"], st["wrow"]
                pswb = psS.tile([P, NB, P], dt.float32, tag="pswb")
                for s in range(NB):
                    nc.tensor.matmul(pswb[:, s], lhsT=ones1,
                                     rhs=wrow[:, s], start=True, stop=True)
                wb = pw.tile([P, NB, P], dt.bfloat16, tag="wb")
                nc.vector.tensor_copy(wb, pswb)

                # weightedT[d, b] = sum_s oT[d, s, b] * w[b, s]
                wt = pwt.tile([P, 8, SB], dt.bfloat16, tag="wt")
                tmpw = pwt.tile([P, 8, SB], dt.bfloat16, tag="tmpw")
                for s in range(NB):
                    a1 = wb[:, None, s, :].to_broadcast((P, 8, SB))
                    if s == 0:
                        nc.vector.tensor_mul(wt, att[:, :, 0, :], a1)
                    else:
                        nc.vector.tensor_mul(tmpw, att[:, :, s, :], a1)
                        nc.vector.tensor_add(wt, wt, tmpw)
                st["wt"] = wt

            def bc_mid(st):
                """refiner layer 1, LN1 -> hb."""
                wt = st["wt"]
                hf = phf.tile([P, 2 * D], dt.bfloat16, tag="hf")
                for n in range(4):
                    ps = psHF.tile([P, 512], dt.float32, tag="psHF")
                    for c in range(8):
                        nc.tensor.matmul(ps, lhsT=wt[:, c],
                                         rhs=r1[:, n, c, :],
                                         start=(c == 0), stop=(c == 7))
                    nc.scalar.copy(hf[:, ts(n, 512)], ps)

                st1 = pw.tile([P, 4, 6], dt.float32, tag="st1")
                for g in range(4):
                    nc.vector.bn_stats(st1[:, g], hf[:, ts(g, 512)])
                mv1 = pw.tile([P, 2], dt.float32, tag="mv1")
                nc.vector.bn_aggr(mv1, st1)
                # relu(LN(x)) = rstd * relu(x - mean): apply only the mean
                # here and fold rstd into the next GEMM's output evac, so
                # Sqrt/reciprocal never block the PE pipeline.
                nmn1 = pw.tile([P, 1], dt.float32, tag="nmn1")
                nc.vector.tensor_scalar(nmn1, mv1[:, 0:1], scalar1=-1.0,
                                        scalar2=None, op0=ALU.mult)
                hb = phf.tile([P, 2 * D], dt.bfloat16, tag="hb")
                nc.vector.tensor_scalar(hb, hf, scalar1=nmn1, scalar2=0.0,
                                        op0=ALU.add, op1=ALU.max)
                sd1 = pw.tile([P, 1], dt.float32, tag="sd1")
                nc.scalar.activation(sd1, mv1[:, 1:2], AF.Sqrt, bias=epst)
                rstd1 = pw.tile([P, 1], dt.float32, tag="rstd1")
                nc.vector.reciprocal(rstd1, sd1)
                st["hb"] = hb
                st["rstd1"] = rstd1

            def bc_back1(st):
                """hb transposes -> hT."""
                hb = st["hb"]
                hT = phT.tile([P, 16, P], dt.bfloat16, tag="hT")
                nc.sync.dma_start_transpose(hT, hb)
                st["hT"] = hT

            def bc_back2(st):
                """refiner layer 2, LN2, store."""
                b0, hT = st["b0"], st["hT"]
                of = pout.tile([P, D], dt.float32, tag="of")
                for n in range(2):
                    ps = psHF.tile([P, 512], dt.float32, tag="psHF")
                    for c in range(16):
                        nc.tensor.matmul(ps, lhsT=hT[:, c],
                                         rhs=r2[:, n, c, :],
                                         start=(c == 0), stop=(c == 15))
                    # deferred LN1 rstd scaling (see bc_mid)
                    nc.scalar.mul(of[:, ts(n, 512)], ps, st["rstd1"])

                st2 = pw.tile([P, 2, 6], dt.float32, tag="st2")
                for g in range(2):
                    nc.vector.bn_stats(st2[:, g], of[:, ts(g, 512)])
                mv2 = pw.tile([P, 2], dt.float32, tag="mv2")
                nc.vector.bn_aggr(mv2, st2)
                sd2 = pw.tile([P, 1], dt.float32, tag="sd2")
                nc.scalar.activation(sd2, mv2[:, 1:2], AF.Sqrt, bias=epst)
                rstd2 = pw.tile([P, 1], dt.float32, tag="rstd2")
                nc.vector.reciprocal(rstd2, sd2)
                fo = pout.tile([P, D], dt.float32, tag="fo")
                nc.vector.tensor_scalar(fo, of, scalar1=mv2[:, 0:1],
                                        scalar2=rstd2, op0=ALU.subtract,
                                        op1=ALU.mult)
                nc.scalar.dma_start(out_d[b0:b0 + SB, :], fo)

            # Pipeline: gate broadcast + weighted sum + refiner for block
            # k-1 and the refiner tail for block k-2 are interleaved with
            # block k's front, so no PE instruction waits on a fresh
            # cross-engine chain.
            prev = None   # block k-1: has w, needs broadcast/wt/mid
            prev2 = None  # block k-2: has hb, needs back1/back2
            for blk in range(nblocks):
                if prev is not None:
                    bc_wrow(prev)
                if prev2 is not None:
                    bc_back1(prev2)
                st = bc_front(blk)
                if prev is not None:
                    bc_wb_wt(prev)
                if prev2 is not None:
                    bc_back2(prev2)
                if prev is not None:
                    bc_mid(prev)
                prev2 = prev
                prev = st
            bc_wrow(prev)
            bc_back1(prev2)
            bc_wb_wt(prev)
            bc_mid(prev)
            bc_back2(prev2)
            bc_back1(prev)
            bc_back2(prev)

    nc.compile()
    return nc


def _chunk_major(wT, n_chunks, width):
    """[K, M] contraction-major weight -> [128, n_chunks, K/128, width]
    (per-partition contiguous; row d of wT maps to (c, p) = (d//128,
    d%128))."""
    K, M = wT.shape
    assert M == n_chunks * width
    a = wT.reshape(K // 128, 128, n_chunks, width)
    return np.ascontiguousarray(a.transpose(1, 2, 0, 3))


def _prep_host_inputs(inputs):
    """Transpose/scale/cast weights, shard x. Returns per-core in_maps."""
    import ml_dtypes
    bf16 = ml_dtypes.bfloat16

    x = _np32(inputs["x"])
    W = _np32(inputs["in_proj_w"]).copy()
    W[:D] *= np.float32(1.0 / np.sqrt(HD))
    # permute the v-section output features from (h, hd) to (hd, h) order
    # so the kernel's o accumulation gets step-1 innermost operands;
    # wg1/r1 rows are permuted identically below to compensate
    vperm = (np.arange(D) % H) * HD + (np.arange(D) // H)
    wqkvT_full = np.ascontiguousarray(W.T)
    wqkvT_full[:, 2 * D:3 * D] = wqkvT_full[:, 2 * D + vperm]
    wqkvT = _chunk_major(wqkvT_full, 6, 512).astype(bf16)
    # fold out_w into wg1 (per branch) and r1 (softmax gate weights sum
    # to 1, so out-proj commutes with the gate-weighted sum of branches)
    ow = _np32(inputs["out_w"])
    wg1 = _np32(inputs["wg1_w"])
    wg1T_full = np.concatenate(
        [(wg1[:, s * D:(s + 1) * D] @ ow).T[vperm] for s in range(NB)],
        axis=0)
    # fp8 DoubleRow layout: rows permuted to the oT tile's (c, s) order,
    # scaled by G1S, pair-of-128-chunks contraction structure
    G1S = 64.0
    wg1_perm = wg1T_full.reshape(NB, 8, P, D).transpose(1, 0, 2, 3)
    w8 = np.clip(wg1_perm.reshape(24, P, D) * G1S, -240.0, 240.0)
    wg1T = np.ascontiguousarray(
        w8.reshape(12, 2, P, 2, 512).transpose(2, 3, 0, 1, 4)
    ).astype(ml_dtypes.float8_e4m3fn)
    r1T = _chunk_major((_np32(inputs["r1_w"]) @ ow).T[vperm],
                       4, 512).astype(bf16)
    wg2T = _chunk_major(_np32(inputs["wg2_w"]).T, 1, 512)[:, 0].astype(bf16)
    wg3T = np.ascontiguousarray(
        _np32(inputs["wg3_w"]).T.reshape(4, P, NB).transpose(1, 0, 2)
    ).astype(bf16)
    r2T = _chunk_major(_np32(inputs["r2_w"]).T, 2, 512).astype(bf16)

    in_maps = []
    nblk = BC // SB
    for c in range(NCORES):
        xc = x[c * BC:(c + 1) * BC]                      # [BC, 3, 1024]
        # [blk, p, c, s, b] so each partition's block-load is contiguous
        x5 = xc.reshape(nblk, SB, NB, 8, P).transpose(0, 4, 3, 2, 1)
        xTc = np.ascontiguousarray(x5).astype(bf16)
        in_maps.append({
            "xT": xTc, "WqkvT": wqkvT, "Wg1T": wg1T,
            "Wg2T": wg2T, "Wg3T": wg3T, "R1T": r1T, "R2T": r2T,
        })
    return in_maps


def _trivial_params(inputs):
    """True iff all biases are zero and LN gains are one (the reference's
    setup_inputs always produces this)."""
    zeros = ["in_proj_b", "out_b", "wg1_b", "wg2_b", "wg3_b", "r1_b", "r2_b",
             "ln1_b", "ln2_b"]
    ones = ["ln1_g", "ln2_g"]
    for k in zeros:
        if np.any(_np32(inputs[k]) != 0.0):
            return False
    for k in ones:
        if np.any(_np32(inputs[k]) != 1.0):
            return False
    return True


def _reference_np(inputs):
    """Plain numpy fallback (only used if bias/gain assumptions fail)."""
    x = _np32(inputs["x"])
    ipw, ipb = _np32(inputs["in_proj_w"]), _np32(inputs["in_proj_b"])
    ow, ob = _np32(inputs["out_w"]), _np32(inputs["out_b"])
    qkv = np.einsum("bsd,ed->bse", x, ipw) + ipb
    q, k, v = np.split(qkv, 3, axis=-1)
    q = q.reshape(B, NB, H, HD)
    k = k.reshape(B, NB, H, HD)
    v = v.reshape(B, NB, H, HD)
    s = np.einsum("bqhd,bkhd->bhqk", q, k) / np.sqrt(np.float32(HD))
    s = s - s.max(-1, keepdims=True)
    e = np.exp(s)
    a = e / e.sum(-1, keepdims=True)
    o = np.einsum("bhqk,bkhd->bqhd", a, v).reshape(B, NB, D)
    att = np.einsum("bsd,ed->bse", o, ow) + ob

    def ln(t, g, bsh):
        m = t.mean(-1, keepdims=True)
        vv = np.square(t - m).mean(-1, keepdims=True)
        return (t - m) / np.sqrt(vv + EPS) * g + bsh

    flat = att.reshape(B, NB * D)
    h = np.maximum(flat @ _np32(inputs["wg1_w"]).T + _np32(inputs["wg1_b"]), 0)
    h = np.maximum(h @ _np32(inputs["wg2_w"]).T + _np32(inputs["wg2_b"]), 0)
    lg = h @ _np32(inputs["wg3_w"]).T + _np32(inputs["wg3_b"])
    lg = lg - lg.max(-1, keepdims=True)
    el = np.exp(lg)
    wgt = el / el.sum(-1, keepdims=True)
    weighted = np.einsum("bsd,bs->bd", att, wgt)
    h = weighted @ _np32(inputs["r1_w"]).T + _np32(inputs["r1_b"])
    h = np.maximum(ln(h, _np32(inputs["ln1_g"]), _np32(inputs["ln1_b"])), 0)
    out = h @ _np32(inputs["r2_w"]).T + _np32(inputs["r2_b"])
    return ln(out, _np32(inputs["ln2_g"]), _np32(inputs["ln2_b"]))


def _get_nc():
    if "nc" not in _CACHE:
        _CACHE["nc"] = _build_program(BC)
    return _CACHE["nc"]


def run_on_cores(in_maps, trace=False, **kw):
    from concourse.bass_utils import run_bass_kernel_spmd
    nc = _get_nc()
    return run_bass_kernel_spmd(nc, in_maps, core_ids=list(range(NCORES)),
                                trace=trace, **kw)


def kernel(**inputs):
    if not _trivial_params(inputs):
        return _reference_np(inputs)
    in_maps = _prep_host_inputs(inputs)
    res = run_on_cores(in_maps)
    out = np.concatenate([res.results[c]["out"] for c in range(NCORES)],
                         axis=0)
    return np.ascontiguousarray(out.astype(np.float32))
